# revision 1
# baseline (speedup 1.0000x reference)
"""v6: dedup + consecutive-vocab PAIRING of h-row gathers.

After np.unique, ~79% of distinct tokens have their successor v+1 also
present. H rows are contiguous in DRAM, so one 128B descriptor fetches
h-rows for BOTH tokens of a (v, v+1) pair: a paired h-block serves 256
tokens (2 output blocks) with ONE h-gather instruction instead of two.
Per-core instructions drop 10795 -> 10545 (~2.3%).

All h-gathers uniformly read 128B (rows v, v+1; H host-padded by one row
so v = VOCAB-1 stays in bounds); single blocks ignore the second row.
Block schedule: h-block i < nblkp covers out-blocks 2i (A=v), 2i+1
(B=v+1) with offset columns 0..16 / 16..32 of the 32-int h row; h-block
nblkp+j covers out-block 2*nblkp+j (cols 0..16).
"""

import numpy as np

VOCAB = 1_000_000
SIZE = 262_144
CHUNK = 8
NCHUNKS = 8
N = 1_048_576
DIM = CHUNK * NCHUNKS

NCORES = 8
P = 128
HBUF = 4
OBUF = 4
SPAR = 8

# capacities for the reference setup_inputs() batch (recomputed at runtime)
NBLKP = 250
NBLKS = 135


def build_kernel(nblkp=NBLKP, nblks=NBLKS):
    import concourse.bass as bass
    import concourse.mybir as mybir
    from concourse.bass import IndirectOffsetOnAxis
    import contextlib

    nhb = nblkp + nblks          # h-blocks (one h-gather each)
    nout = 2 * nblkp + nblks     # output blocks of 128 tokens
    # jobs[b] = (h-block index, offset column base)
    jobs = [(i, cb) for i in range(nblkp) for cb in (0, 16)]
    jobs += [(nblkp + j, 0) for j in range(nblks)]
    first_b = {}
    last_b = {}
    for b, (hb, cb) in enumerate(jobs):
        first_b.setdefault(hb, b)
        last_b[hb] = b

    nc = bass.Bass(trn_type="TRN2")
    x_t = nc.dram_tensor("x", [P, nhb], mybir.dt.int32, kind="ExternalInput")
    h_t = nc.dram_tensor(
        "h", [VOCAB + 1, 2 * NCHUNKS], mybir.dt.int32, kind="ExternalInput"
    )
    t0_t = nc.dram_tensor(
        "t0", [1, SIZE + CHUNK], mybir.dt.float32, kind="ExternalInput"
    )
    t1_t = nc.dram_tensor(
        "t1", [1, SIZE + CHUNK], mybir.dt.float32, kind="ExternalInput"
    )
    out_t = nc.dram_tensor(
        "out", [nout * P, DIM], mybir.dt.float32, kind="ExternalOutput"
    )

    out_v = out_t[:].rearrange("(b p) d -> b p d", p=P)

    with contextlib.ExitStack() as ctx:
        x_sb = ctx.enter_context(nc.sbuf_tensor("x_sb", [P, nhb], mybir.dt.int32))
        h_sb = ctx.enter_context(
            nc.sbuf_tensor("h_sb", [P, HBUF, 2 * 2 * NCHUNKS], mybir.dt.int32)
        )
        o_sb = ctx.enter_context(
            nc.sbuf_tensor("o_sb", [P, OBUF, DIM], mybir.dt.float32)
        )
        sem_x = ctx.enter_context(nc.semaphore("sem_x"))
        sem_h = [ctx.enter_context(nc.semaphore(f"sem_h{s}")) for s in range(HBUF)]
        sem_s0 = [ctx.enter_context(nc.semaphore(f"sem_s0{s}")) for s in range(SPAR)]
        sem_s1 = [ctx.enter_context(nc.semaphore(f"sem_s1{s}")) for s in range(SPAR)]
        sem_st = [ctx.enter_context(nc.semaphore(f"sem_st{s}")) for s in range(OBUF)]

        nc.sync.dma_start(x_sb[:], x_t[:]).then_inc(sem_x, 16)

        for L in range(nout + 3):
            # ---- Pool: h-gather when out-block L opens a new h-block ----
            if L < nout:
                hb, cb = jobs[L]
                if first_b[hb] == L:
                    if L == 0:
                        nc.gpsimd.wait_ge(sem_x, 16)
                    if hb >= HBUF:
                        # slot reuse: ALL out-blocks of h-block hb-HBUF must
                        # have finished their t1 reads of the slot
                        prev = hb - HBUF
                        for k in {first_b[prev], last_b[prev]}:
                            nc.gpsimd.wait_ge(
                                sem_s1[k % SPAR], 128 * (k // SPAR + 1)
                            )
                    nc.gpsimd.indirect_dma_start(
                        out=h_sb[:, hb % HBUF, :],
                        out_offset=None,
                        in_=h_t[:],
                        in_offset=IndirectOffsetOnAxis(
                            ap=x_sb[:, hb : hb + 1], axis=0
                        ),
                    ).then_inc(sem_h[hb % HBUF], 16)

            # ---- Pool: t0 slice batch for out-block b0 = L-1 ----
            b0 = L - 1
            if 0 <= b0 < nout:
                hb0, cb0 = jobs[b0]
                nc.gpsimd.wait_ge(sem_h[hb0 % HBUF], 16 * (hb0 // HBUF + 1))
                if b0 >= OBUF:
                    k = b0 - OBUF
                    nc.gpsimd.wait_ge(sem_st[k % OBUF], 16 * (k // OBUF + 1))
                for c in range(8):
                    nc.gpsimd.indirect_dma_start(
                        out=o_sb[:, b0 % OBUF, c * 8 : (c + 1) * 8],
                        out_offset=None,
                        in_=t0_t[:],
                        in_offset=IndirectOffsetOnAxis(
                            ap=h_sb[:, hb0 % HBUF, cb0 + c : cb0 + c + 1], axis=1
                        ),
                    ).then_inc(sem_s0[b0 % SPAR], 16)

            # ---- Pool: t1 slice batch + CCE add for b1 = L-2 ----
            b1 = L - 2
            if 0 <= b1 < nout:
                hb1, cb1 = jobs[b1]
                nc.gpsimd.wait_ge(sem_s0[b1 % SPAR], 128 * (b1 // SPAR + 1))
                for c in range(8):
                    nc.gpsimd.indirect_dma_start(
                        out=o_sb[:, b1 % OBUF, c * 8 : (c + 1) * 8],
                        out_offset=None,
                        in_=t1_t[:],
                        in_offset=IndirectOffsetOnAxis(
                            ap=h_sb[:, hb1 % HBUF, cb1 + 8 + c : cb1 + 8 + c + 1],
                            axis=1,
                        ),
                        compute_op=mybir.AluOpType.add,
                    ).then_inc(sem_s1[b1 % SPAR], 16)

            # ---- SP: store out-block L-3 ----
            sb = L - 3
            if 0 <= sb < nout:
                nc.sync.wait_ge(sem_s1[sb % SPAR], 128 * (sb // SPAR + 1))
                nc.sync.dma_start(out_v[sb], o_sb[:, sb % OBUF, :]).then_inc(
                    sem_st[sb % OBUF], 16
                )

        for s in range(OBUF):
            ns = len([k for k in range(nout) if k % OBUF == s])
            if ns:
                nc.sync.wait_ge(sem_st[s], ns * 16)
    return nc


def plan(x):
    """Dedup + pair consecutive vocab values; shard over cores.

    Returns (xw [NCORES, P, nhb] h-block values, order_vals
    [NCORES, nout*P] vocab value per output row (-1 = padding),
    inv, uniq_len, nblkp, nblks)."""
    x = np.ascontiguousarray(np.asarray(x).astype(np.int32))
    uniq, inv = np.unique(x, return_inverse=True)
    U = len(uniq)
    d = np.diff(uniq) == 1
    pairs, singles = [], []
    i = 0
    while i < U - 1:
        if d[i]:
            pairs.append(uniq[i])
            i += 2
        else:
            singles.append(uniq[i])
            i += 1
    if i == U - 1:
        singles.append(uniq[i])
    pairs = np.asarray(pairs, dtype=np.int32)
    singles = np.asarray(singles, dtype=np.int32)
    pch = np.array_split(pairs, NCORES)
    sch = np.array_split(singles, NCORES)
    nblkp = -(-max(len(c) for c in pch) // P)
    nblks = -(-max(len(c) for c in sch) // P)
    nhb = nblkp + nblks
    nout = 2 * nblkp + nblks
    xw = np.zeros((NCORES, P, nhb), dtype=np.int32)
    order_vals = np.full((NCORES, nout * P), -1, dtype=np.int64)
    for k in range(NCORES):
        pv = np.zeros(nblkp * P, dtype=np.int32)
        pv[: len(pch[k])] = pch[k]
        sv = np.zeros(nblks * P, dtype=np.int32)
        sv[: len(sch[k])] = sch[k]
        # h-block i partition p -> pair/single index i*P + p
        xw[k, :, :nblkp] = pv.reshape(nblkp, P).T
        xw[k, :, nblkp:] = sv.reshape(nblks, P).T
        ov = order_vals[k]
        npr, nsg = len(pch[k]), len(sch[k])
        for i in range(nblkp):
            lo, hi = i * P, min((i + 1) * P, npr)
            if lo >= npr:
                break
            n = hi - lo
            ov[2 * i * P : 2 * i * P + n] = pch[k][lo:hi]
            ov[(2 * i + 1) * P : (2 * i + 1) * P + n] = pch[k][lo:hi] + 1
        base = 2 * nblkp * P
        ov[base : base + nsg] = sch[k]
    return xw, order_vals, inv, uniq, nblkp, nblks


def kernel(table0, table1, h0, h1, x):
    from concourse.bass_utils import run_bass_kernel_spmd

    xw, order_vals, inv, uniq, nblkp, nblks = plan(x)
    H = np.concatenate([h0, h1], axis=1).astype(np.int32)
    H = np.ascontiguousarray(np.concatenate([H, H[:1]], axis=0))  # +1 pad row
    t0 = np.ascontiguousarray(
        np.concatenate([table0, table0[:CHUNK]]).astype(np.float32)
    ).reshape(1, SIZE + CHUNK)
    t1 = np.ascontiguousarray(
        np.concatenate([table1, table1[:CHUNK]]).astype(np.float32)
    ).reshape(1, SIZE + CHUNK)
    nc = build_kernel(nblkp, nblks)
    in_maps = [
        {"x": xw[k], "h": H, "t0": t0, "t1": t1} for k in range(NCORES)
    ]
    res = run_bass_kernel_spmd(nc, in_maps, core_ids=list(range(NCORES)))
    uniq_out = np.empty((len(uniq), DIM), dtype=np.float32)
    for k in range(NCORES):
        rows = res.results[k]["out"]
        ov = order_vals[k]
        valid = ov >= 0
        uniq_out[np.searchsorted(uniq, ov[valid])] = rows[valid]
    return uniq_out[inv]

